# revision 12
# baseline (speedup 1.0000x reference)
"""DocumentDualEmbedder pooling kernel for Trainium2 (Bass/Tile).

Per doc b (B=64 docs, S=2048 tokens, D=256 dims):
    w     = idf[chunk[b]];  wn = w / sum(w)
    out[b] = concat(sum_s wn[s]*x[s],            # idf-weighted mean  [D]
                    max_s x[s], min_s x[s],      # max / min pool     [D each]
                    sqrt(S/(S-1)*(E[x^2]-mu^2))) # unbiased std       [D]

Distribution: pure data parallel over the batch dim -- each of the 8
NeuronCores processes 8 docs, no collectives.  Host prep (inside
kernel()): bf16 cast + device-layout swizzle of encoding, idf gather +
normalization into a packed [1/S | w_norm] stationary table.

Device structure per core (s = q*16 + t, q = partition, t = chunk):
  - docs stream in pairs: per-doc 1MB HWDGE loads (8KB/partition
    contiguous), per-doc ACT squares into a separate tile so the
    x-stream matmuls and the max/min trees depend only on the DMA.
  - mean/mu: 16 matmuls per doc, lhsT = [1/S | w_norm] (bf16), rhs = x_t;
    E[x^2]: 16 matmuls per doc, lhsT = [1/S], rhs = sq_t.  Both accumulate
    in per-half PSUM tiles that the tail reads directly (no per-doc
    drains; all PSUM engine reads are partition-base 0).
  - max/min: DVE pairwise trees in bf16 (2x mode), fused across the doc
    pair; partition reduction per half via TAIL_MODE:
      "transpose": PE transpose (via identity) -> PSUM -> DVE free-dim
                   reduce -> PE transpose back -> ACT drain -> out DMA.
      "gpsimd":    gpsimd partition_all_reduce(max) (min via negate).
  - std: musq = Square(mu) on ACT, var = E - musq on DVE, sqrt on ACT.
  - Small/output DMAs ride the SP HWDGE ring; tails run at high
    scheduler priority so the first half's epilogue overlaps the second
    half's main loop.
"""

import numpy as np
import ml_dtypes

import concourse.bass as bass
import concourse.bacc as bacc
import concourse.tile as tile
from concourse import mybir, bass_isa
from concourse.bass_utils import run_bass_kernel_spmd

B, S, D, V = 64, 2048, 256, 32000
NCORES = 8
BL = B // NCORES          # 8 docs per core
T = 16                    # chunks per doc
P = 128                   # partitions
HB = BL // 2              # half-batch (4 docs)
NP = BL // 2              # doc pairs
F32 = mybir.dt.float32
BF16 = mybir.dt.bfloat16
STD_SCALE = float(S) / float(S - 1)

TAIL_MODE = "transpose"


def build_bass(reps: int = 1):
    nc = bacc.Bacc("TRN2", target_bir_lowering=False, debug=False)
    xarr_d = nc.dram_tensor("xarr", [P, BL * T * D], BF16, kind="ExternalInput")
    wl_d = nc.dram_tensor("wl", [P, BL * T * 2], BF16, kind="ExternalInput")
    ident_d = nc.dram_tensor("ident", [P, P], BF16, kind="ExternalInput")
    out_d = nc.dram_tensor("out", [BL, 4 * D], F32, kind="ExternalOutput")

    with tile.TileContext(nc) as tc:
      for _rep in range(reps):
        with (
            tc.tile_pool(name="singles", bufs=1) as singles,
            tc.tile_pool(name="xpool", bufs=3) as xpool,
            tc.tile_pool(name="treepool", bufs=2) as treepool,
            tc.tile_pool(name="tailpool", bufs=1) as tailpool,
            tc.tile_pool(name="pstat", bufs=1, space="PSUM") as pstat,
            tc.tile_pool(name="ptrans", bufs=1, space="PSUM") as ptrans,
            tc.tile_pool(name="prps", bufs=1, space="PSUM") as prps,
        ):
            wl = singles.tile([P, BL, T, 2], BF16)
            nc.scalar.dma_start(out=wl[:], in_=wl_d[:, :])
            ident = None
            if TAIL_MODE == "transpose":
                ident = singles.tile([P, P], BF16)
                nc.scalar.dma_start(out=ident[:], in_=ident_d[:, :])

            mall = [singles.tile([P, HB, D], BF16, name=f"mall{h}", tag=f"mall{h}")
                    for h in range(2)]
            nall = [singles.tile([P, HB, D], BF16, name=f"nall{h}", tag=f"nall{h}")
                    for h in range(2)]

            def tail_stats(h, ps, psB, b0):
                # ps row 0 = mu, row 1 = mean (all PSUM reads base-0)
                musq = tailpool.tile([1, HB, D], F32, tag="musq")
                nc.scalar.activation(
                    musq[:], ps[0:1, :, :],
                    mybir.ActivationFunctionType.Square)
                mr2 = tailpool.tile([2, HB, D], F32, tag="mr2")
                nc.scalar.copy(mr2[:], ps[:, :, :])
                nc.sync.dma_start(out=out_d[b0:b0 + HB, 0:D], in_=mr2[1:2, :, :])
                var0 = tailpool.tile([1, HB, D], F32, tag="var0")
                nc.vector.tensor_tensor(
                    var0[:], psB[0:1, :, :], musq[:],
                    op=mybir.AluOpType.subtract)
                stdrow = tailpool.tile([1, HB, D], F32, tag="stdrow")
                nc.scalar.activation(
                    stdrow[:], var0[:], mybir.ActivationFunctionType.Sqrt,
                    scale=STD_SCALE)
                nc.sync.dma_start(
                    out=out_d[b0:b0 + HB, 3 * D:4 * D], in_=stdrow[:])

            def tail_half_transpose(h, ps, psB):
                b0 = h * HB
                for stat, acc, alu in (("mx", mall[h], mybir.AluOpType.max),
                                       ("mn", nall[h], mybir.AluOpType.min)):
                    trp = ptrans.tile([P, 2 * HB, P], BF16, tag=f"trp{stat}")
                    for j in range(HB):
                        for k in range(2):
                            nc.tensor.transpose(
                                trp[:, 2 * j + k, :],
                                acc[:, j, k * P:(k + 1) * P],
                                ident[:],
                            )
                    red = tailpool.tile([P, 2 * HB], BF16, tag=f"red{stat}")
                    nc.vector.tensor_reduce(
                        red[:], trp[:], axis=mybir.AxisListType.X, op=alu)
                    rps = prps.tile([2 * HB, P], BF16, tag="rps")
                    nc.tensor.transpose(rps[:], red[:], ident[:])
                    rsb = tailpool.tile([2 * HB, P], F32, tag=f"rsb{stat}")
                    nc.scalar.copy(rsb[:], rps[:])
                    col = D if stat == "mx" else 2 * D
                    nc.sync.dma_start(
                        out=out_d[b0:b0 + HB, col:col + D], in_=rsb[:])
                tail_stats(h, ps, psB, b0)

            def tail_half_gpsimd(h, ps, psB):
                b0 = h * HB
                nneg = tailpool.tile([P, HB, D], BF16, tag="nneg")
                nc.vector.tensor_scalar_mul(nneg[:], nall[h][:], -1.0)
                mred = tailpool.tile([P, HB, D], F32, tag="mred")
                nc.gpsimd.partition_all_reduce(
                    mred[:], mall[h][:], channels=P,
                    reduce_op=bass_isa.ReduceOp.max)
                nc.sync.dma_start(
                    out=out_d[b0:b0 + HB, D:2 * D], in_=mred[0:1, :, :])
                nred = tailpool.tile([P, HB, D], F32, tag="nred")
                nc.gpsimd.partition_all_reduce(
                    nred[:], nneg[:], channels=P,
                    reduce_op=bass_isa.ReduceOp.max)
                minrow = tailpool.tile([1, HB, D], F32, tag="minrow")
                nc.scalar.mul(minrow[:], nred[0:1, :, :], -1.0)
                nc.sync.dma_start(
                    out=out_d[b0:b0 + HB, 2 * D:3 * D], in_=minrow[:])
                tail_stats(h, ps, psB, b0)

            tail_half = (tail_half_gpsimd if TAIL_MODE == "gpsimd"
                         else tail_half_transpose)

            ps_tiles = {}
            psB_tiles = {}
            for p in range(NP):
                h, pj = divmod(p, NP // 2)      # half, pair-within-half
                b0 = 2 * p
                # x and sq in separate tiles: the x-stream matmuls and the
                # trees depend only on the DMA, not on the squares.
                xt = xpool.tile([P, 2, T, D], BF16, tag="xt")
                sq = xpool.tile([P, 2, T, D], BF16, tag="sqt")
                for dj in range(2):
                    nc.sync.dma_start(
                        out=xt[:, dj, :, :],
                        in_=xarr_d[:, (b0 + dj) * T * D:(b0 + dj + 1) * T * D]
                        .rearrange("q (t d) -> q t d", d=D))
                    nc.scalar.activation(
                        sq[:, dj, :, :], xt[:, dj, :, :],
                        mybir.ActivationFunctionType.Square)

                if pj == 0:
                    ps_tiles[h] = pstat.tile([2, HB, D], F32, name="ps", tag="ps")
                    psB_tiles[h] = pstat.tile([1, HB, D], F32, name="psB", tag="psB")
                ps = ps_tiles[h]
                psB = psB_tiles[h]
                for dj in range(2):
                    b = b0 + dj
                    jj = b - h * HB
                    for t in range(T):
                        nc.tensor.matmul(
                            ps[:, jj, :],
                            lhsT=wl[:, b, t, :],
                            rhs=xt[:, dj, t, :],
                            start=(t == 0),
                            stop=(t == T - 1),
                            skip_group_check=True,
                        )
                    for t in range(T):
                        nc.tensor.matmul(
                            psB[:, jj, :],
                            lhsT=wl[:, b, t, 0:1],
                            rhs=sq[:, dj, t, :],
                            start=(t == 0),
                            stop=(t == T - 1),
                            skip_group_check=True,
                        )

                if p == 0:
                    # first pair: per-doc trees so the DVE starts as soon
                    # as doc 0 lands (it back-pressures the whole kernel)
                    for dj in range(2):
                        x_d = xt[:, dj, :, :]
                        jj = b0 + dj - h * HB
                        for stat, alu, acc in (("mx", mybir.AluOpType.max, mall[h]),
                                               ("mn", mybir.AluOpType.min, nall[h])):
                            t1 = treepool.tile([P, 8, D], BF16, name="t1", tag=f"{stat}1")
                            nc.vector.tensor_tensor(
                                t1[:], x_d[:, 0:8, :], x_d[:, 8:16, :], op=alu)
                            t2 = treepool.tile([P, 4, D], BF16, name="t2", tag=f"{stat}2")
                            nc.vector.tensor_tensor(
                                t2[:], t1[:, 0:4, :], t1[:, 4:8, :], op=alu)
                            t3 = treepool.tile([P, 2, D], BF16, name="t3", tag=f"{stat}3")
                            nc.vector.tensor_tensor(
                                t3[:], t2[:, 0:2, :], t2[:, 2:4, :], op=alu)
                            nc.vector.tensor_tensor(
                                acc[:, jj, :], t3[:, 0, :], t3[:, 1, :], op=alu)
                else:
                    # later pairs: fused across the pair (fewer op inits)
                    x_p = xt[:, :, :, :]
                    jj0 = b0 - h * HB
                    for stat, alu, acc in (("mx", mybir.AluOpType.max, mall[h]),
                                           ("mn", mybir.AluOpType.min, nall[h])):
                        p1 = treepool.tile([P, 2, 8, D], BF16, name="p1", tag=f"p{stat}1")
                        nc.vector.tensor_tensor(
                            p1[:], x_p[:, :, 0:8, :], x_p[:, :, 8:16, :], op=alu)
                        p2 = treepool.tile([P, 2, 4, D], BF16, name="p2", tag=f"p{stat}2")
                        nc.vector.tensor_tensor(
                            p2[:], p1[:, :, 0:4, :], p1[:, :, 4:8, :], op=alu)
                        p3 = treepool.tile([P, 2, 2, D], BF16, name="p3", tag=f"p{stat}3")
                        nc.vector.tensor_tensor(
                            p3[:], p2[:, :, 0:2, :], p2[:, :, 2:4, :], op=alu)
                        nc.vector.tensor_tensor(
                            acc[:, jj0:jj0 + 2, :], p3[:, :, 0, :], p3[:, :, 1, :],
                            op=alu)

                if pj == NP // 2 - 1:
                    with tc.high_priority():
                        tail_half(h, ps_tiles[h], psB_tiles[h])

    nc.finalize()
    return nc


_NC = None


def _get_nc():
    global _NC
    if _NC is None:
        _NC = build_bass()
    return _NC


def make_in_maps(chunk, encoding, idf):
    chunk = np.ascontiguousarray(np.asarray(chunk, dtype=np.int32))
    encoding = np.asarray(encoding, dtype=np.float32)
    idf = np.asarray(idf, dtype=np.float32).reshape(V)
    ident = np.eye(P, dtype=ml_dtypes.bfloat16)
    in_maps = []
    for c in range(NCORES):
        sl = slice(c * BL, (c + 1) * BL)
        # [b, s, d] -> [q, b, t, d], bf16
        xa = encoding[sl].reshape(BL, P, T, D).transpose(1, 0, 2, 3)
        xa = np.ascontiguousarray(xa).astype(ml_dtypes.bfloat16)
        w = idf[chunk[sl]]                          # [BL, S]
        w = w / w.sum(axis=1, keepdims=True)
        wl = np.empty((P, BL, T, 2), dtype=np.float32)
        wl[..., 0] = 1.0 / S
        wl[..., 1] = w.reshape(BL, P, T).transpose(1, 0, 2)
        in_maps.append({
            "xarr": xa.reshape(P, BL * T * D),
            "wl": wl.reshape(P, BL * T * 2).astype(ml_dtypes.bfloat16),
            "ident": ident,
        })
    return in_maps


def kernel(chunk: np.ndarray, encoding: np.ndarray, idf: np.ndarray) -> np.ndarray:
    nc = _get_nc()
    in_maps = make_in_maps(chunk, encoding, idf)
    res = run_bass_kernel_spmd(nc, in_maps, core_ids=list(range(NCORES)))
    out = np.concatenate([res.results[c]["out"] for c in range(NCORES)], axis=0)
    return out.astype(np.float32)


if __name__ == "__main__":
    rng = np.random.default_rng(0)
    chunk = rng.integers(0, V, size=(B, S), dtype=np.int32)
    encoding = rng.standard_normal((B, S, D), dtype=np.float32)
    idf = rng.uniform(1e-3, 1.0, size=(V,)).astype(np.float32)
    out = kernel(chunk=chunk, encoding=encoding, idf=idf)
    print("out", out.shape, out.dtype, out[0, :4])
